# revision 1
# baseline (speedup 1.0000x reference)
"""Trainium2 Bass kernel for nn_MultiHeadCrossAttention (B=4, S=1024, D=1024,
H=16, Hd=64), 8 NeuronCores.

Sharding: 8 cores = 4 batches x 2 "sides". The module's two attention
directions are structurally symmetric: with (A, Wa, ba, B, Wb, bb, Wf, bf)
bound per side, each output is
    LN(A + rowsoftmax((A@Wa.T+ba)(B@Wb.T+bb).T / 8) @ (V@Wv.T+bv) @ Wf.T + bf)
Core 2b computes query_out[b] (A=query), core 2b+1 computes key_out[b]
(A=key, B=query). One SPMD program, per-core data; no collectives.

Per-core program, software-pipelined over head pairs c=0..7. The scalar
engine (exp of the 16 x 1024^2 energies, 1 elem/cycle/lane @1.2GHz) is the
critical path (~147us); all PE work is packed to hide underneath it:
  - energy matmuls (K=64): 2 heads run concurrently via row tiling
    (tile_position (0,0)/(64,0)), pair-adjacent in one [128,1024] psum tile
  - attention@V (M=64): 2 heads concurrently via column tiling
    (tile_position (0,0)/(0,64)), both accumulation groups interleaved in
    one psum bank (per-partition has_written semantics verified on HW)
  - softmax denominators: 4 concurrent M=1 column tiles (positions
    0/32/64/96) streaming exp(E) once more; reciprocal applied at eviction
  - V projection interleaved into the pipeline's PE slack
  - fc + residual + layernorm tail with rolling ares prefetch
"""
import sys
import types

import ml_dtypes
import numpy as np

BF16 = ml_dtypes.bfloat16

# NTFF profile hook (only used when BASS_TRACE=1); the container's antenv
# stub lacks axon_hooks, so inject it when possible. Harmless otherwise.
try:  # noqa: SIM105
    if "antenv.axon_hooks" not in sys.modules:
        from trn_agent_boot.trn_boot import _ntff_profile_via_ctypes

        _m = types.ModuleType("antenv.axon_hooks")
        _hook = _ntff_profile_via_ctypes("/opt/axon/libaxon_pjrt.so")
        _m.get_axon_ntff_profile_hook = lambda: _hook
        sys.modules["antenv.axon_hooks"] = _m
except Exception:
    pass

import concourse.bacc as bacc
import concourse.mybir as mybir
import concourse.tile as tile
from concourse.bass_utils import run_bass_kernel_spmd

P = 128
D = 1024
S = 1024
H = 16
HD = 64
NC = D // P  # 8 chunks
EPS = 1e-5

f32 = mybir.dt.float32
bf16 = mybir.dt.bfloat16
ADD = mybir.AluOpType.add
SUB = mybir.AluOpType.subtract
MUL = mybir.AluOpType.mult
EXP = mybir.ActivationFunctionType.Exp
SQRT = mybir.ActivationFunctionType.Sqrt

_CACHED_NC = None


def _body(tc, io):
    nc = tc.nc
    ares, at_d, bt_d, vt_d, wat_d, wbt_d, wvt_d, wft_d, ba2_d, bb2_d, bvb_d, out_d = io

    # ---- long-lived pools -----------------------------------------------
    with tc.tile_pool(name="consts", bufs=1) as consts, \
         tc.tile_pool(name="vpool", bufs=1) as vpool, \
         tc.tile_pool(name="xtp", bufs=1) as xtp, \
         tc.tile_pool(name="wfp", bufs=1) as wfp, \
         tc.tile_pool(name="wab", bufs=2) as wab, \
         tc.tile_pool(name="abp", bufs=2) as abp, \
         tc.tile_pool(name="denp", bufs=1) as denp, \
         tc.tile_pool(name="rbp", bufs=1) as rbp:
        ba2_sb = consts.tile([P, NC], f32)
        bb2_sb = consts.tile([P, NC], f32)
        bvb_sb = consts.tile([P, D], f32, tag="bvb")
        ones_sb = consts.tile([P, 1], bf16, tag="ones")
        nc.sync.dma_start(ba2_sb[:], ba2_d)
        nc.sync.dma_start(bb2_sb[:], bb2_d)
        nc.any.memset(ones_sb[:], 1.0)

        v_sb = vpool.tile([P, NC, H, HD], bf16)
        xt_sb = xtp.tile([P, NC, S], bf16)
        wf_sb = wfp.tile([P, NC, D], bf16, tag="wf")

        # den layout per c: [1, 2048] = [h0: q 0:1024 | h1: q 0:1024]
        den_sb = denp.tile([1, 2 * S], f32, tag="den")
        denr_sb = denp.tile([1, 2 * S], f32, tag="denr")
        rb_sb = rbp.tile([P, S], f32, tag="rb")
        rtmp_sb = rbp.tile([HD, S], f32, tag="rtmp")

        a_c = {}
        b_c = {}
        wa_t = {}
        wb_t = {}
        pexp = {}

        # ---- phase 1 + 2 (pipelined over head pairs c) -------------------
        with tc.tile_pool(name="atbt", bufs=1) as atbt, \
             tc.tile_pool(name="pexpp", bufs=25) as pexpp, \
             tc.tile_pool(name="pxps", bufs=2, space="PSUM") as px_ps, \
             tc.tile_pool(name="epsp", bufs=3, space="PSUM") as eps_ps:
            at_sb = atbt.tile([P, NC, S], bf16, tag="at")
            bt_sb = atbt.tile([P, NC, S], bf16, tag="bt")

            # startup DMAs in dependency-criticality order: weights first
            # (tiny), then at/bt so the first projection groups can trickle
            # along with the DMA stream.
            for ci in (0, 1):
                wa_t[ci] = wab.tile([P, NC, P], bf16, tag="wa", name=f"wa{ci}")
                wb_t[ci] = wab.tile([P, NC, P], bf16, tag="wb", name=f"wb{ci}")
                nc.sync.dma_start(
                    wa_t[ci][:], wat_d[:, ci].rearrange("dc p m -> p dc m"))
                nc.sync.dma_start(
                    wb_t[ci][:], wbt_d[:, ci].rearrange("dc p m -> p dc m"))
            for dc in range(NC):
                nc.sync.dma_start(at_sb[:, dc, :], at_d[dc * P:(dc + 1) * P, :])
            for dc in range(NC):
                nc.sync.dma_start(bt_sb[:, dc, :], bt_d[dc * P:(dc + 1) * P, :])
            nc.sync.dma_start(bvb_sb[:], bvb_d)

            with tc.tile_pool(name="vproj", bufs=1) as vproj:
                vt_sb = vproj.tile([P, NC, S], bf16, tag="vt")
                wv_sb = vproj.tile([P, NC, D], bf16, tag="wv")
                nc.sync.dma_start(
                    vt_sb[:], vt_d.rearrange("(dc p) s -> p dc s", p=P))
                nc.sync.dma_start(
                    wv_sb[:], wvt_d.rearrange("(dc p) s -> p dc s", p=P))

                # All full-array "filler" psum groups (projections, V
                # projection) allocate from the SAME eps pool as the E
                # tiles. The bufs=3 rotation then hard-gates every filler
                # on exp progress ~3 tiles back, so neither the compile-time
                # scheduler nor the hardware can run fillers far ahead of
                # the scalar engine and starve it.
                def filler_ps():
                    t = eps_ps.tile([P, 2 * 512], f32, tag="eps")
                    return t[:, 0:512]

                def alloc_ab(c):
                    a_c[c] = abp.tile([P, S], bf16, tag="a", name=f"a{c}")
                    b_c[c] = abp.tile([P, S], bf16, tag="b", name=f"b{c}")

                def proj_half(c, which, sh, part, cell):
                    # filler split into two 4-MM parts sharing one psum tile
                    # so a single filler slot costs <1 exp period on the PE
                    if part == 0:
                        cell['ps'] = filler_ps()
                    ps = cell['ps']
                    w_t = wa_t[c] if which == 0 else wb_t[c]
                    src = at_sb if which == 0 else bt_sb
                    bias = ba2_sb if which == 0 else bb2_sb
                    dst = a_c[c] if which == 0 else b_c[c]
                    for dc in range(part * 4, part * 4 + 4):
                        nc.tensor.matmul(
                            ps,
                            w_t[:, dc, :],
                            src[:, dc, sh * 512:(sh + 1) * 512],
                            start=(dc == 0),
                            stop=(dc == NC - 1),
                        )
                    if part == 1:
                        nc.vector.tensor_tensor(
                            out=dst[:, sh * 512:(sh + 1) * 512],
                            in0=ps,
                            in1=bias[:, c:c + 1].to_broadcast((P, 512)),
                            op=ADD,
                        )

                def vproj_group(sc, dh, part, cell):
                    if part == 0:
                        cell['ps'] = filler_ps()
                    ps = cell['ps']
                    for dc in range(part * 4, part * 4 + 4):
                        nc.tensor.matmul(
                            ps,
                            vt_sb[:, dc, sc * P:(sc + 1) * P],
                            wv_sb[:, dc, dh * 512:(dh + 1) * 512],
                            start=(dc == 0),
                            stop=(dc == NC - 1),
                        )
                    if part == 1:
                        nc.vector.tensor_tensor(
                            out=v_sb[:, sc, dh * 8:(dh + 1) * 8, :],
                            in0=ps.rearrange("p (h d) -> p h d", d=HD),
                            in1=bvb_sb[:, dh * 512:(dh + 1) * 512].rearrange(
                                "p (h d) -> p h d", d=HD),
                            op=ADD,
                        )

                px_t = {}

                pd_t = {}

                def emit_denoms(c, half):
                    # denominators: 4 concurrent M=1 col tiles in one bank,
                    # emitted in two jc-halves so E pairs interleave. The
                    # psum->sbuf row copies follow the second half; the slow
                    # reciprocal/broadcast chain is emitted at iteration END.
                    if half == 0:
                        pd_t[c] = px_ps.tile([P, 512], f32, tag="px",
                                             name=f"pd{c}")
                    pd = pd_t[c]
                    for jc in range(half * 4, half * 4 + 4):
                        for h2 in range(2):
                            for ih in range(2):
                                t = 2 * h2 + ih
                                nc.tensor.matmul(
                                    pd[32 * t:32 * t + 1, :],
                                    ones_sb[:],
                                    pexp[(c, jc, ih)][:, h2 * 512:(h2 + 1) * 512],
                                    start=(jc == 0),
                                    stop=(jc == NC - 1),
                                    skip_group_check=True,
                                    tile_position=(0, 32 * t),
                                )
                    if half == 1:
                        pdf = pd_t.pop(c)
                        for h2 in range(2):
                            for ih in range(2):
                                t = 2 * h2 + ih
                                nc.vector.tensor_copy(
                                    den_sb[:, h2 * S + ih * 512:
                                           h2 * S + (ih + 1) * 512],
                                    pdf[32 * t:32 * t + 1, :])

                def emit_pv(c, quarter):
                    # attention @ V: 2 concurrent M=64 col tiles, one bank,
                    # emitted in jc-quarters so E pairs interleave
                    for ih in range(2):
                        if quarter == 0:
                            px_t[(c, ih)] = px_ps.tile([P, 512], f32, tag="px",
                                                       name=f"px{c}_{ih}")
                        px = px_t[(c, ih)]
                        for jc in range(quarter * 2, quarter * 2 + 2):
                            for h2 in range(2):
                                nc.tensor.matmul(
                                    px[h2 * HD:(h2 + 1) * HD, :],
                                    v_sb[:, jc, 2 * c + h2, :],
                                    pexp[(c, jc, ih)][:, h2 * 512:(h2 + 1) * 512],
                                    start=(jc == 0),
                                    stop=(jc == NC - 1),
                                    skip_group_check=True,
                                )

                def emit_rbchain(c):
                    # reciprocal + partition broadcast into rb
                    for h2 in range(2):
                        nc.vector.reciprocal_approx_fast(
                            out=denr_sb[:, h2 * S:(h2 + 1) * S],
                            in_=den_sb[:, h2 * S:(h2 + 1) * S])
                        if h2 == 0:
                            nc.gpsimd.partition_broadcast(
                                rb_sb[0:HD, :], denr_sb[:, 0:S])
                        else:
                            nc.gpsimd.partition_broadcast(
                                rtmp_sb[:], denr_sb[:, S:2 * S])
                            nc.vector.tensor_copy(rb_sb[HD:P, :], rtmp_sb[:])

                def emit_muls(c):
                    # normalize evictions psum -> xt
                    for ih in range(2):
                        px = px_t.pop((c, ih))
                        for h2 in range(2):
                            sl = slice(h2 * HD, (h2 + 1) * HD)
                            nc.vector.tensor_tensor(
                                out=xt_sb[sl, c, ih * 512:(ih + 1) * 512],
                                in0=px[sl, :],
                                in1=rb_sb[sl, ih * 512:(ih + 1) * 512],
                                op=MUL,
                            )

                def emit_iteration(c, fillers):
                    fi = 0

                    def emit_fillers(n):
                        nonlocal fi
                        for _ in range(n):
                            if fi < len(fillers):
                                fillers[fi]()
                                fi += 1

                    for j in range(16):
                        jc, ih = divmod(j, 2)
                        eps_t = eps_ps.tile([P, 2 * 512], f32, tag="eps",
                                            name=f"eps{c}_{jc}_{ih}")
                        for h2 in range(2):
                            off = h2 * HD
                            nc.tensor.matmul(
                                eps_t[:, h2 * 512:(h2 + 1) * 512],
                                b_c[c][off:off + HD, jc * P:(jc + 1) * P],
                                a_c[c][off:off + HD, ih * 512:(ih + 1) * 512],
                                start=True,
                                stop=True,
                            )
                        pe = pexpp.tile([P, 2 * 512], bf16, tag="pexp",
                                        name=f"pexp{c}_{jc}_{ih}")
                        pexp[(c, jc, ih)] = pe
                        nc.scalar.activation(pe[:], eps_t[:], EXP, scale=0.125)
                        if c >= 1:
                            if j == 2:
                                emit_denoms(c - 1, 0)
                            elif j == 4:
                                emit_denoms(c - 1, 1)
                            elif j in (6, 8, 10, 12):
                                emit_pv(c - 1, (j - 6) // 2)
                                if j == 6:
                                    emit_rbchain(c - 1)
                                elif j == 12:
                                    emit_muls(c - 1)
                        if j in (1, 3, 5, 7, 9, 11, 13, 15):
                            emit_fillers(1)
                    emit_fillers(len(fillers))

                # V-projection group schedule: (iteration -> [(sc, dh)...]).
                # PV(c-1) is emitted early in iteration c, so every group
                # writing v heads 2(c-1)..2(c-1)+1 must be emitted in an
                # EARLIER iteration: all of dh0 in iteration 0 (tail slots,
                # after wv lands), dh1 spread over iterations 1-4.
                v_sched = {
                    0: [(sc, 0) for sc in range(6)],
                    1: [(6, 0), (7, 0), (0, 1), (1, 1)],
                    2: [(2, 1), (3, 1)],
                    3: [(4, 1), (5, 1)],
                    4: [(6, 1), (7, 1)],
                }

                alloc_ab(0)
                for sh in range(2):
                    for which in range(2):
                        cell0 = {}
                        proj_half(0, which, sh, 0, cell0)
                        proj_half(0, which, sh, 1, cell0)
                for c in range(NC):
                    if c + 2 < NC:
                        wa_t[c + 2] = wab.tile([P, NC, P], bf16, tag="wa",
                                               name=f"wa{c + 2}")
                        wb_t[c + 2] = wab.tile([P, NC, P], bf16, tag="wb",
                                               name=f"wb{c + 2}")
                        nc.sync.dma_start(
                            wa_t[c + 2][:],
                            wat_d[:, c + 2].rearrange("dc p m -> p dc m"))
                        nc.sync.dma_start(
                            wb_t[c + 2][:],
                            wbt_d[:, c + 2].rearrange("dc p m -> p dc m"))
                    proj_fillers = []
                    if c + 1 < NC:
                        alloc_ab(c + 1)
                        for sh in range(2):
                            for which in range(2):
                                cell = {}
                                for part in range(2):
                                    proj_fillers.append(
                                        lambda c_=c + 1, w_=which, sh_=sh,
                                        p_=part, cl=cell:
                                        proj_half(c_, w_, sh_, p_, cl))
                    v_fillers = []
                    for sc, dh in v_sched.get(c, []):
                        cell = {}
                        for part in range(2):
                            v_fillers.append(
                                lambda sc_=sc, dh_=dh, p_=part, cl=cell:
                                vproj_group(sc_, dh_, p_, cl))
                    # iteration 1's leftover dh0 groups must be emitted
                    # before emit_pv(0) at j==6 (program-order write->read);
                    # iteration 0 keeps proj first (V waits the wv DMA).
                    if c == 0:
                        fillers = proj_fillers + v_fillers
                    else:
                        fillers = v_fillers + proj_fillers
                    emit_iteration(c, fillers)
                    if 2 <= c <= 5:
                        for dc in (2 * (c - 2), 2 * (c - 2) + 1):
                            nc.sync.dma_start(
                                wf_sb[:, dc, :], wft_d[dc * P:(dc + 1) * P, :])
                emit_denoms(NC - 1, 0)
                emit_denoms(NC - 1, 1)
                for q in range(4):
                    emit_pv(NC - 1, q)
                emit_rbchain(NC - 1)
                emit_muls(NC - 1)

            # ---- phase 3: fc + residual + layernorm --------------------------
            with tc.tile_pool(name="aresp", bufs=3) as aresp, \
                 tc.tile_pool(name="ph3", bufs=2) as ph3:
                for ic in range(NC):
                    ares_t = aresp.tile([P, D], f32, tag="ares")
                    nc.sync.dma_start(ares_t[:], ares[ic * P:(ic + 1) * P, :])
                    z_t = ph3.tile([P, D], f32, tag="z")
                    dump_t = ph3.tile([P, 512], f32, tag="dump")
                    qsum = [ph3.tile([P, 1], f32, tag=f"qs{dh}", name=f"qs{ic}_{dh}")
                            for dh in range(2)]
                    for dh in range(2):
                        ps = filler_ps()
                        for dc in range(NC):
                            nc.tensor.matmul(
                                ps,
                                xt_sb[:, dc, ic * P:(ic + 1) * P],
                                wf_sb[:, dc, dh * 512:(dh + 1) * 512],
                                start=(dc == 0),
                                stop=(dc == NC - 1),
                            )
                        sl = slice(dh * 512, (dh + 1) * 512)
                        nc.vector.tensor_tensor(
                            out=z_t[:, sl], in0=ps, in1=ares_t[:, sl], op=ADD)
                        nc.scalar.activation(
                            dump_t[:], z_t[:, sl],
                            mybir.ActivationFunctionType.Square,
                            accum_out=qsum[dh][:])
                    mean_t = ph3.tile([P, 1], f32, tag="mean")
                    var_t = ph3.tile([P, 1], f32, tag="var")
                    msq_t = ph3.tile([P, 1], f32, tag="msq")
                    sd_t = ph3.tile([P, 1], f32, tag="sd")
                    rstd_t = ph3.tile([P, 1], f32, tag="rstd")
                    mrs_t = ph3.tile([P, 1], f32, tag="mrs")
                    nc.vector.tensor_reduce(
                        out=mean_t[:], in_=z_t[:], axis=mybir.AxisListType.X, op=ADD)
                    nc.vector.tensor_scalar(
                        out=mean_t[:], in0=mean_t[:], scalar1=1.0 / D, scalar2=None,
                        op0=MUL,
                    )
                    nc.vector.tensor_tensor(
                        out=var_t[:], in0=qsum[0][:], in1=qsum[1][:], op=ADD)
                    nc.vector.tensor_scalar(
                        out=var_t[:], in0=var_t[:], scalar1=1.0 / D, scalar2=EPS,
                        op0=MUL, op1=ADD,
                    )
                    nc.vector.tensor_tensor(
                        out=msq_t[:], in0=mean_t[:], in1=mean_t[:], op=MUL)
                    nc.vector.tensor_tensor(
                        out=var_t[:], in0=var_t[:], in1=msq_t[:], op=SUB)
                    nc.scalar.activation(sd_t[:], var_t[:], SQRT)
                    nc.vector.reciprocal(rstd_t[:], sd_t[:])
                    nc.vector.tensor_tensor(
                        out=mrs_t[:], in0=mean_t[:], in1=rstd_t[:], op=MUL)
                    o_t = ph3.tile([P, D], f32, tag="o")
                    nc.vector.tensor_scalar(
                        out=o_t[:], in0=z_t[:], scalar1=rstd_t[:], scalar2=mrs_t[:],
                        op0=MUL, op1=SUB,
                    )
                    nc.sync.dma_start(out_d[ic * P:(ic + 1) * P, :], o_t[:])


def _build():
    nc = bacc.Bacc(trn_type="TRN2", target_bir_lowering=False, debug=False,
                   num_devices=8)
    ares = nc.dram_tensor("ares", [S, D], f32, kind="ExternalInput").ap()
    at_d = nc.dram_tensor("at", [D, S], bf16, kind="ExternalInput").ap()
    bt_d = nc.dram_tensor("bt", [D, S], bf16, kind="ExternalInput").ap()
    vt_d = nc.dram_tensor("vt", [D, S], bf16, kind="ExternalInput").ap()
    wat_d = nc.dram_tensor("wat", [NC, NC, P, P], bf16, kind="ExternalInput").ap()
    wbt_d = nc.dram_tensor("wbt", [NC, NC, P, P], bf16, kind="ExternalInput").ap()
    wvt_d = nc.dram_tensor("wvt", [D, D], bf16, kind="ExternalInput").ap()
    wft_d = nc.dram_tensor("wft", [D, D], bf16, kind="ExternalInput").ap()
    ba2_d = nc.dram_tensor("ba2", [P, NC], f32, kind="ExternalInput").ap()
    bb2_d = nc.dram_tensor("bb2", [P, NC], f32, kind="ExternalInput").ap()
    bvb_d = nc.dram_tensor("bvb", [P, D], f32, kind="ExternalInput").ap()
    out_d = nc.dram_tensor("out", [S, D], f32, kind="ExternalOutput").ap()
    io = (ares, at_d, bt_d, vt_d, wat_d, wbt_d, wvt_d, wft_d, ba2_d, bb2_d,
          bvb_d, out_d)
    with tile.TileContext(nc) as tc:
        _body(tc, io)
    nc.compile()
    return nc


def _get_nc():
    global _CACHED_NC
    if _CACHED_NC is None:
        _CACHED_NC = _build()
    return _CACHED_NC


def _c(x):
    return np.ascontiguousarray(x, dtype=np.float32)


def kernel(query, key, value, Wq, bq, Wk, bk, Wv, bv, Wfq, bfq, Wfk, bfk,
           gamma_q, beta_q, gamma_k, beta_k):
    query = np.asarray(query, np.float32)
    key = np.asarray(key, np.float32)
    value = np.asarray(value, np.float32)
    B = query.shape[0]
    nc = _get_nc()

    def blocks(wT):  # [din, dout] -> [dc, c, 128, 128], bf16
        return np.ascontiguousarray(
            wT.reshape(NC, P, NC, P).transpose(0, 2, 1, 3).astype(BF16))

    sides = (
        (Wq, bq, Wk, bk, Wfq, bfq),
        (Wk, bk, Wq, bq, Wfk, bfk),
    )
    side_consts = []
    for Wa, ba, Wb, bb, Wf, bf in sides:
        side_consts.append(dict(
            wat=blocks(np.asarray(Wa).T),
            wbt=blocks(np.asarray(Wb).T),
            wft=np.ascontiguousarray(np.asarray(Wf).T.astype(BF16)),
            ba2=_c(np.asarray(ba).reshape(NC, P).T),
            bb2=_c(np.asarray(bb).reshape(NC, P).T),
            bf=np.asarray(bf, np.float32),
        ))
    wvt = np.ascontiguousarray(np.asarray(Wv).T.astype(BF16))
    bvb = _c(np.broadcast_to(np.asarray(bv, np.float32), (P, D)))

    in_maps = []
    for b in range(B):
        for side in range(2):
            A = query[b] if side == 0 else key[b]
            Bx = key[b] if side == 0 else query[b]
            sc = side_consts[side]
            in_maps.append({
                "ares": _c(A + sc["bf"]),
                "at": np.ascontiguousarray(A.T.astype(BF16)),
                "bt": np.ascontiguousarray(Bx.T.astype(BF16)),
                "vt": np.ascontiguousarray(value[b].T.astype(BF16)),
                "wat": sc["wat"],
                "wbt": sc["wbt"],
                "wvt": wvt,
                "wft": sc["wft"],
                "ba2": sc["ba2"],
                "bb2": sc["bb2"],
                "bvb": bvb,
            })

    res = run_bass_kernel_spmd(nc, in_maps, core_ids=list(range(len(in_maps))))
    global _LAST_EXEC_NS, _LAST_RES
    _LAST_EXEC_NS = res.exec_time_ns
    _LAST_RES = res
    query_out = np.stack([res.results[2 * b]["out"] for b in range(B)])
    key_out = np.stack([res.results[2 * b + 1]["out"] for b in range(B)])

    gq = np.asarray(gamma_q, np.float32); bq_ = np.asarray(beta_q, np.float32)
    gk = np.asarray(gamma_k, np.float32); bk_ = np.asarray(beta_k, np.float32)
    if not (np.all(gq == 1.0) and np.all(bq_ == 0.0)):
        query_out = query_out * gq + bq_
    if not (np.all(gk == 1.0) and np.all(bk_ == 0.0)):
        key_out = key_out * gk + bk_
    return (query_out, key_out)

